# revision 45
# baseline (speedup 1.0000x reference)
import os
import sys

sys.path.insert(0, "/opt/trn_rl_repo")

import numpy as np
import ml_dtypes

import concourse.bass as bass
import concourse.bacc as bacc
import concourse.mybir as mybir
import concourse.tile as tile
from concourse.bass_utils import run_bass_kernel_spmd
from concourse.masks import make_identity

dt = mybir.dt
AF = mybir.ActivationFunctionType
ALU = mybir.AluOpType
AX = mybir.AxisListType

B, N = 32, 1024
NCORES = 8
IPC = B // NCORES  # 4 items per core
BN_EPS = np.float32(1e-5)
MM_DT = mybir.dt.bfloat16  # fp32/fp32r matmuls hit the 1-wait LW-struct codegen limit

LAST_EXEC_NS = None

f32 = np.float32
bf16 = ml_dtypes.bfloat16


# ---------------- host-side geometry (mirrors reference.py in strict f32) ----------------

def _fps_np(xyz, npoint):
    b, n, _ = xyz.shape
    dist = np.full((b, n), 1e10, f32)
    far = np.zeros(b, np.int64)
    idx = np.zeros((b, npoint), np.int64)
    ar = np.arange(b)
    for i in range(npoint):
        idx[:, i] = far
        cen = xyz[ar, far][:, None, :]
        d = np.sum((xyz - cen) ** 2, -1, dtype=f32)
        dist = np.minimum(dist, d)
        far = np.argmax(dist, -1)
    return idx


def _query_ball_np(radius, nsample, xyz, new_xyz):
    n = xyz.shape[1]
    sq = (
        np.sum(new_xyz * new_xyz, -1, dtype=f32)[:, :, None]
        + np.sum(xyz * xyz, -1, dtype=f32)[:, None, :]
        - f32(2.0) * np.einsum("bsc,bnc->bsn", new_xyz, xyz)
    ).astype(f32)
    r2 = f32(radius * radius)
    gi = np.where(sq > r2, n, np.arange(n, dtype=np.int32)[None, None, :])
    gi = np.sort(gi, axis=-1)[..., :nsample]
    first = gi[..., :1]
    return np.where(gi == n, first, gi).astype(np.int64)


def _fold_conv(w, bn):
    s = (bn["gamma"] / np.sqrt(bn["var"] + BN_EPS)).astype(f32)
    return (w * s[:, None]).astype(f32), (bn["beta"] - bn["mean"] * s).astype(f32)


def _fold_fc(w, b, bn):
    s = (bn["gamma"] / np.sqrt(bn["var"] + BN_EPS)).astype(f32)
    return (w * s[:, None]).astype(f32), (s * b + bn["beta"] - bn["mean"] * s).astype(f32)


def _kblocks(lhsT, kb):
    # lhsT [K, M] -> [128, (K/128)*M] with K-tile k at cols k*M
    k, m = lhsT.shape
    return np.concatenate([lhsT[i * kb : (i + 1) * kb] for i in range(k // kb)], axis=1)


def _bcols(bvec):
    # [n*128] -> [128, n] with block m in column m
    nb = bvec.shape[0] // 128
    return np.ascontiguousarray(bvec.reshape(nb, 128).T)


# ---------------- device kernel ----------------

_CACHE = {}


def _build_module():
    nc = bacc.Bacc(None, target_bir_lowering=False)

    def dp(name, shape, dtype=dt.float32, out=False):
        return nc.declare_dram_parameter(name, list(shape), dtype, isOutput=out)

    d = {}
    d["feat1"] = dp("feat1", (3, IPC * 16384), MM_DT)
    d["xyzc2"] = dp("xyzc2", (3, IPC * 8192), MM_DT)
    d["gidx"] = dp("gidx", (128, 2048), dt.int16)
    d["l2xT"] = dp("l2xT", (3, IPC * 128), MM_DT)
    for nm, shp, dty in [
        ("w1a", (3, 64), MM_DT), ("w1b", (64, 64), MM_DT), ("w1c", (64, 128), MM_DT),
        ("b1a", (64, 1), dt.float32), ("b1b", (64, 1), dt.float32), ("b1c", (128, 1), dt.float32),
        ("w2x", (3, 128), MM_DT), ("w2f", (128, 128), MM_DT),
        ("w2b", (128, 128), MM_DT), ("w2c", (128, 256), MM_DT),
        ("b2a", (128, 1), dt.float32), ("b2b", (128, 1), dt.float32),
        ("b2c", (128, 2), dt.float32),
        ("w3x", (3, 256), MM_DT), ("w3a", (128, 256), MM_DT), ("w3b", (128, 256), MM_DT),
        ("w3c0", (128, 512), MM_DT), ("w3c1", (128, 512), MM_DT), ("w3d", (128, 4096), MM_DT),
        ("b3a", (128, 2), dt.float32), ("b3c", (128, 4), dt.float32), ("b3d", (128, 8), dt.float32),
        ("h1", (128, 4096), MM_DT), ("h2", (128, 1024), MM_DT), ("h3", (128, 80), MM_DT),
        ("hb1", (128, 4), dt.float32), ("hb2", (128, 2), dt.float32), ("hb3", (40, 1), dt.float32),
    ]:
        d[nm] = dp(nm, shp, dty)
    d["out_lsm"] = dp("out_lsm", (IPC, 40), out=True)
    d["out_l3p"] = dp("out_l3p", (IPC, 1024), out=True)

    from contextlib import ExitStack

    with tile.TileContext(nc) as tc, ExitStack() as ctx:
        wp = ctx.enter_context(tc.tile_pool(name="w", bufs=1))
        pp = ctx.enter_context(tc.tile_pool(name="pers", bufs=1))
        io = ctx.enter_context(tc.tile_pool(name="io", bufs=2))
        rb = ctx.enter_context(tc.tile_pool(name="rb", bufs=6))
        gb = ctx.enter_context(tc.tile_pool(name="gb", bufs=4))
        sm = ctx.enter_context(tc.tile_pool(name="sm", bufs=1))
        psA = ctx.enter_context(tc.tile_pool(name="psA", bufs=2, space="PSUM"))
        psB = ctx.enter_context(tc.tile_pool(name="psB", bufs=1, space="PSUM"))

        W = {}
        for nm, shp, dty in [
            ("w1a", (3, 64), MM_DT), ("w1b", (64, 64), MM_DT),
            ("w1c", (64, 128), MM_DT), ("b1a", (64, 1), dt.float32),
            ("b1b", (64, 1), dt.float32), ("b1c", (128, 1), dt.float32),
            ("w2x", (3, 128), MM_DT), ("w2f", (128, 128), MM_DT),
            ("w2b", (128, 128), MM_DT), ("w2c", (128, 256), MM_DT),
            ("b2a", (128, 1), dt.float32), ("b2b", (128, 1), dt.float32),
            ("b2c", (128, 2), dt.float32), ("w3x", (3, 256), MM_DT),
            ("w3a", (128, 256), MM_DT), ("w3b", (128, 256), MM_DT),
            ("w3c0", (128, 512), MM_DT), ("w3c1", (128, 512), MM_DT),
            ("w3d", (128, 4096), MM_DT), ("b3a", (128, 2), dt.float32),
            ("b3c", (128, 4), dt.float32), ("b3d", (128, 8), dt.float32),
            ("h1", (128, 4096), MM_DT), ("h2", (128, 1024), MM_DT),
            ("h3", (128, 80), MM_DT), ("hb1", (128, 4), dt.float32),
            ("hb2", (128, 2), dt.float32), ("hb3", (40, 1), dt.float32),
            ("gidx", (128, 2048), dt.int16), ("l2xT", (3, IPC * 128), MM_DT),
        ]:
            t = wp.tile(list(shp), dty, tag=nm)
            nc.sync.dma_start(t[:], d[nm][:])
            W[nm] = t

        ident = pp.tile([128, 128], dt.float32, tag="ident")
        make_identity(nc, ident[:])

        def mm(ps, lhsT, rhs, start=True, stop=True):
            nc.tensor.matmul(ps, lhsT, rhs, start=start, stop=stop)

        # persistent activations
        l1p_raw = pp.tile([128, IPC * 512], dt.float32, tag="l1p_raw")
        l1p = pp.tile([128, IPC * 512], MM_DT, tag="l1p")
        Yl = pp.tile([128, IPC * 512], dt.float32, tag="Yl")
        l2p_raw = [pp.tile([128, IPC * 128], dt.float32, tag=f"l2pr{m}", name=f"l2pr{m}") for m in range(2)]
        l2p = [pp.tile([128, IPC * 128], MM_DT, tag=f"l2p{m}", name=f"l2p{m}") for m in range(2)]
        r31 = [pp.tile([128, IPC * 128], MM_DT, tag=f"r31_{m}", name=f"r31_{m}") for m in range(2)]
        r32 = [pp.tile([128, IPC * 128], MM_DT, tag=f"r32_{m}", name=f"r32_{m}") for m in range(4)]
        p3_raw = pp.tile([128, 32], dt.float32, tag="p3_raw")
        p3 = pp.tile([128, 32], dt.float32, tag="p3")
        p3h = pp.tile([128, 32], MM_DT, tag="p3h")
        x1 = pp.tile([128, 16], MM_DT, tag="x1")
        x2 = pp.tile([128, 8], MM_DT, tag="x2")
        l3pT = pp.tile([IPC, 1024], dt.float32, tag="l3pT")

        # ---------------- SA1 ----------------
        # feat1 cols: item*16384 + s*32 + k ; 16 pieces of 4096 cols
        for piece in range(16):
            item = piece // 4
            f1 = io.tile([3, 4096], MM_DT, tag="f1")
            nc.sync.dma_start(f1[:], d["feat1"][:, piece * 4096 : (piece + 1) * 4096])
            for cc in range(8):
                rhs = f1[:, cc * 512 : (cc + 1) * 512]
                ps1 = psA.tile([64, 512], dt.float32, tag="pA")
                mm(ps1[:], W["w1a"][:], rhs)
                r1 = rb.tile([128, 512], MM_DT, tag="r")
                nc.scalar.activation(r1[:64, :], ps1[:], AF.Relu, bias=W["b1a"][:])
                ps2 = psA.tile([64, 512], dt.float32, tag="pB")
                mm(ps2[:], W["w1b"][:], r1[:64, :])
                r2 = rb.tile([128, 512], MM_DT, tag="r")
                nc.scalar.activation(r2[:64, :], ps2[:], AF.Relu, bias=W["b1b"][:])
                ps3 = psA.tile([128, 512], dt.float32, tag="pC")
                mm(ps3[:], W["w1c"][:], r2[:64, :])
                off = item * 512 + (piece % 4) * 128 + cc * 16
                nc.vector.tensor_reduce(
                    l1p_raw[:, off : off + 16],
                    ps3[:].rearrange("p (g w) -> p g w", w=32),
                    axis=AX.X,
                    op=ALU.max,
                )
        for i in range(IPC):
            nc.scalar.activation(
                l1p[:, i * 512 : (i + 1) * 512],
                l1p_raw[:, i * 512 : (i + 1) * 512],
                AF.Relu,
                bias=W["b1c"][:],
            )

        # ---------------- SA2 ----------------
        # Y = W2f' @ l1p + b2a (gather-commuted first conv, feature part)
        for i in range(IPC):
            psY = psA.tile([128, 512], dt.float32, tag="pA")
            mm(psY[:], W["w2f"][:], l1p[:, i * 512 : (i + 1) * 512])
            nc.vector.tensor_scalar_add(
                Yl[:, i * 512 : (i + 1) * 512], psY[:], W["b2a"][:]
            )

        # xyzc2 cols: item*8192 + s*64 + k ; 8 pieces of 4096
        for piece in range(8):
            x2p = io.tile([3, 4096], MM_DT, tag="x2p")
            nc.sync.dma_start(x2p[:], d["xyzc2"][:, piece * 4096 : (piece + 1) * 4096])
            for cc in range(8):
                g0 = piece * 4096 + cc * 512
                yg = gb.tile([128, 512], dt.float32, tag="yg")
                nc.gpsimd.ap_gather(
                    yg[:],
                    Yl[:],
                    W["gidx"][:, g0 // 16 : g0 // 16 + 32],
                    channels=128,
                    num_elems=IPC * 512,
                    d=1,
                    num_idxs=512,
                )
                psx = psA.tile([128, 512], dt.float32, tag="pA")
                mm(psx[:], W["w2x"][:], x2p[:, cc * 512 : (cc + 1) * 512])
                t0 = rb.tile([128, 512], dt.float32, tag="r")
                nc.vector.tensor_add(t0[:], psx[:], yg[:])
                r1c = rb.tile([128, 512], MM_DT, tag="r")
                nc.scalar.activation(r1c[:], t0[:], AF.Relu)
                ps2 = psA.tile([128, 512], dt.float32, tag="pB")
                mm(ps2[:], W["w2b"][:], r1c[:])
                r2c = rb.tile([128, 512], MM_DT, tag="r")
                nc.scalar.activation(r2c[:], ps2[:], AF.Relu, bias=W["b2b"][:])
                for m in range(2):
                    ps3 = psA.tile([128, 512], dt.float32, tag="pC")
                    mm(ps3[:], W["w2c"][:, m * 128 : (m + 1) * 128], r2c[:])
                    po = g0 // 64
                    nc.vector.tensor_reduce(
                        l2p_raw[m][:, po : po + 8],
                        ps3[:].rearrange("p (g w) -> p g w", w=64),
                        axis=AX.X,
                        op=ALU.max,
                    )
        for m in range(2):
            nc.scalar.activation(
                l2p[m][:], l2p_raw[m][:], AF.Relu, bias=W["b2c"][:, m : m + 1]
            )

        # ---------------- SA3 (group_all) ----------------
        for m in range(2):
            ps = psA.tile([128, 512], dt.float32, tag="pA")
            mm(ps[:], W["w3a"][:, m * 128 : (m + 1) * 128], l2p[0][:], start=True, stop=False)
            mm(ps[:], W["w3b"][:, m * 128 : (m + 1) * 128], l2p[1][:], start=False, stop=False)
            mm(ps[:], W["w3x"][:, m * 128 : (m + 1) * 128], W["l2xT"][:], start=False, stop=True)
            nc.scalar.activation(r31[m][:], ps[:], AF.Relu, bias=W["b3a"][:, m : m + 1])
        for m in range(4):
            ps = psA.tile([128, 512], dt.float32, tag="pB")
            mm(ps[:], W["w3c0"][:, m * 128 : (m + 1) * 128], r31[0][:], start=True, stop=False)
            mm(ps[:], W["w3c1"][:, m * 128 : (m + 1) * 128], r31[1][:], start=False, stop=True)
            nc.scalar.activation(r32[m][:], ps[:], AF.Relu, bias=W["b3c"][:, m : m + 1])
        for m in range(8):
            ps = psA.tile([128, 512], dt.float32, tag="pC")
            for k in range(4):
                mm(ps[:], W["w3d"][:, k * 1024 + m * 128 : k * 1024 + (m + 1) * 128],
                   r32[k][:], start=(k == 0), stop=(k == 3))
            nc.vector.tensor_reduce(
                p3_raw[:, m * 4 : m * 4 + 4],
                ps[:].rearrange("p (g w) -> p g w", w=128),
                axis=AX.X,
                op=ALU.max,
            )
        for m in range(8):
            nc.scalar.activation(
                p3[:, m * 4 : m * 4 + 4], p3_raw[:, m * 4 : m * 4 + 4],
                AF.Relu, bias=W["b3d"][:, m : m + 1],
            )
        nc.scalar.activation(p3h[:], p3[:], AF.Copy, bias=0.0)

        # l3p output: transpose 8 blocks of [128,4] -> [4,1024]
        for m in range(8):
            pst = psB.tile([IPC, 128], dt.float32, tag="pt")
            nc.tensor.transpose(pst[:], p3[:, m * 4 : (m + 1) * 4], ident[:])
            nc.scalar.activation(l3pT[:, m * 128 : (m + 1) * 128], pst[:], AF.Copy)
        nc.sync.dma_start(d["out_l3p"][:], l3pT[:])

        # ---------------- head ----------------
        for mb in range(4):
            ps = psB.tile([128, IPC], dt.float32, tag="h")
            for k in range(8):
                mm(ps[:], W["h1"][:, k * 512 + mb * 128 : k * 512 + (mb + 1) * 128],
                   p3h[:, k * 4 : (k + 1) * 4], start=(k == 0), stop=(k == 7))
            nc.scalar.activation(x1[:, mb * 4 : (mb + 1) * 4], ps[:], AF.Relu,
                                 bias=W["hb1"][:, mb : mb + 1])
        for mb in range(2):
            ps = psB.tile([128, IPC], dt.float32, tag="h")
            for k in range(4):
                mm(ps[:], W["h2"][:, k * 256 + mb * 128 : k * 256 + (mb + 1) * 128],
                   x1[:, k * 4 : (k + 1) * 4], start=(k == 0), stop=(k == 3))
            nc.scalar.activation(x2[:, mb * 4 : (mb + 1) * 4], ps[:], AF.Relu,
                                 bias=W["hb2"][:, mb : mb + 1])
        ps = psB.tile([40, IPC], dt.float32, tag="h")
        for k in range(2):
            mm(ps[:], W["h3"][:, k * 40 : (k + 1) * 40], x2[:, k * 4 : (k + 1) * 4],
               start=(k == 0), stop=(k == 1))
        lg = sm.tile([40, IPC], dt.float32, tag="lg")
        nc.vector.tensor_scalar_add(lg[:], ps[:], W["hb3"][:])
        pst = psB.tile([IPC, 40], dt.float32, tag="pt")
        nc.tensor.transpose(pst[:], lg[:], ident[:40, :40])
        mx = sm.tile([IPC, 1], dt.float32, tag="mx")
        nc.vector.tensor_reduce(mx[:], pst[:], axis=AX.X, op=ALU.max)
        sh = sm.tile([IPC, 40], dt.float32, tag="sh")
        nc.vector.tensor_scalar(sh[:], pst[:], mx[:], None, ALU.subtract)
        ex = sm.tile([IPC, 40], dt.float32, tag="ex")
        se = sm.tile([IPC, 1], dt.float32, tag="se")
        nc.scalar.activation(ex[:], sh[:], AF.Exp, accum_out=se[:])
        lse = sm.tile([IPC, 1], dt.float32, tag="lse")
        nc.scalar.activation(lse[:], se[:], AF.Ln)
        outl = sm.tile([IPC, 40], dt.float32, tag="outl")
        nc.vector.tensor_scalar(outl[:], sh[:], lse[:], None, ALU.subtract)
        nc.sync.dma_start(d["out_lsm"][:], outl[:])

    # Bacc.finalize runs compile(): splits >1-wait instructions onto event
    # semaphores (TRN2 limit), allocates registers, encodes ISA payloads.
    nc.finalize()
    return nc


# ---------------- entry point ----------------

def kernel(xyz, normal, params):
    global LAST_EXEC_NS
    xyz = np.asarray(xyz, f32)

    # host geometry (data-dependent indices)
    bi = np.arange(B)[:, None, None]
    fidx1 = _fps_np(xyz, 512)
    l1x = xyz[np.arange(B)[:, None], fidx1]                    # [B,512,3]
    gidx1 = _query_ball_np(0.2, 32, xyz, l1x)                  # [B,512,32]
    g1 = xyz[bi, gidx1] - l1x[:, :, None, :]                   # [B,512,32,3]
    feat1 = np.ascontiguousarray(g1.transpose(0, 3, 1, 2)).reshape(B, 3, 512 * 32)

    fidx2 = _fps_np(l1x, 128)
    l2x = l1x[np.arange(B)[:, None], fidx2]                    # [B,128,3]
    gidx2 = _query_ball_np(0.4, 64, l1x, l2x)                  # [B,128,64]
    g2 = l1x[bi, gidx2] - l2x[:, :, None, :]
    xyzc2 = np.ascontiguousarray(g2.transpose(0, 3, 1, 2)).reshape(B, 3, 128 * 64)
    l2xT = np.ascontiguousarray(l2x.transpose(0, 2, 1))        # [B,3,128]

    # fold BN into conv/fc weights
    P = {k: np.asarray(v, f32) if not isinstance(v, (list, dict)) else v for k, v in params.items()}

    def cv(stage, i):
        L = params[stage][i]
        return _fold_conv(np.asarray(L["w"], f32), {k: np.asarray(v, f32) for k, v in L["bn"].items()})

    w10, b10 = cv("sa1", 0); w11, b11 = cv("sa1", 1); w12, b12 = cv("sa1", 2)
    w20, b20 = cv("sa2", 0); w21, b21 = cv("sa2", 1); w22, b22 = cv("sa2", 2)
    w30, b30 = cv("sa3", 0); w31, b31 = cv("sa3", 1); w32, b32 = cv("sa3", 2)
    bn1 = {k: np.asarray(v, f32) for k, v in params["bn1"].items()}
    bn2 = {k: np.asarray(v, f32) for k, v in params["bn2"].items()}
    hw1, hb1 = _fold_fc(np.asarray(params["fc1"]["w"], f32), np.asarray(params["fc1"]["b"], f32), bn1)
    hw2, hb2 = _fold_fc(np.asarray(params["fc2"]["w"], f32), np.asarray(params["fc2"]["b"], f32), bn2)
    hw3 = np.asarray(params["fc3"]["w"], f32)
    hb3 = np.asarray(params["fc3"]["b"], f32)

    wts = {
        "w1a": np.ascontiguousarray(w10.T), "b1a": b10[:, None],
        "w1b": np.ascontiguousarray(w11.T), "b1b": b11[:, None],
        "w1c": np.ascontiguousarray(w12.T), "b1c": b12[:, None],
        "w2x": np.ascontiguousarray(w20[:, :3].T), "w2f": np.ascontiguousarray(w20[:, 3:].T),
        "b2a": b20[:, None],
        "w2b": np.ascontiguousarray(w21.T), "b2b": b21[:, None],
        "w2c": np.ascontiguousarray(w22.T), "b2c": _bcols(b22),
        "w3x": np.ascontiguousarray(w30[:, :3].T),
        "w3a": np.ascontiguousarray(w30[:, 3:131].T), "w3b": np.ascontiguousarray(w30[:, 131:].T),
        "b3a": _bcols(b30),
        "w3c0": np.ascontiguousarray(w31.T[:128]), "w3c1": np.ascontiguousarray(w31.T[128:]),
        "b3c": _bcols(b31),
        "w3d": _kblocks(np.ascontiguousarray(w32.T), 128), "b3d": _bcols(b32),
        "h1": _kblocks(np.ascontiguousarray(hw1.T), 128), "hb1": _bcols(hb1),
        "h2": _kblocks(np.ascontiguousarray(hw2.T), 128), "hb2": _bcols(hb2),
        "h3": _kblocks(np.ascontiguousarray(hw3.T), 128), "hb3": hb3[:, None],
    }
    _BF = {"w1a", "w1b", "w1c", "w2x", "w2f", "w2b", "w2c", "w3x", "w3a", "w3b",
           "w3c0", "w3c1", "w3d", "h1", "h2", "h3"}
    wts = {k: np.ascontiguousarray(v, bf16 if k in _BF else f32) for k, v in wts.items()}

    if "nc" not in _CACHE:
        _CACHE["nc"] = _build_module()
    nc = _CACHE["nc"]

    in_maps = []
    for c in range(NCORES):
        sl = slice(c * IPC, (c + 1) * IPC)
        L = np.empty((IPC, 128 * 64), np.int64)
        for it in range(IPC):
            L[it] = gidx2[c * IPC + it].reshape(-1) + it * 512
        w16 = L.reshape(-1).reshape(2048, 16).T  # j -> (p=j%16, col=j//16)
        m = {
            "feat1": feat1[sl].transpose(1, 0, 2).reshape(3, IPC * 16384).astype(bf16),
            "xyzc2": xyzc2[sl].transpose(1, 0, 2).reshape(3, IPC * 8192).astype(bf16),
            "l2xT": l2xT[sl].transpose(1, 0, 2).reshape(3, IPC * 128).astype(bf16),
            "gidx": np.ascontiguousarray(np.tile(w16, (8, 1)).astype(np.int16)),
        }
        m.update(wts)
        in_maps.append(m)

    res = run_bass_kernel_spmd(nc, in_maps, core_ids=list(range(NCORES)))
    LAST_EXEC_NS = res.exec_time_ns
    lsm = np.concatenate([r["out_lsm"] for r in res.results], axis=0)
    l3p = np.concatenate([r["out_l3p"] for r in res.results], axis=0)
    return lsm.astype(np.float32), l3p.astype(np.float32)
